# revision 7
# baseline (speedup 1.0000x reference)
"""Trainium2 Bass kernel for CoreML alignment (duration -> frame expansion).

Math: each output frame f belongs to exactly one token l (the alignment
matrix is one-hot per frame: starts[l] <= f < cum_dur[l]), so

    en[:, c, f]  = d[:, tok(f), c]      (gather of d rows)
    asr[:, c, f] = t_en[:, c, tok(f)]   (gather of t_en columns)

and every column f >= T_a = sum(dur) is exactly zero.

Strategy (frame-sharded, no collectives):
  - Host computes cum_dur / frame->token indices (tiny metadata) and pads
    d / t_en^T with one zero row; inactive frames index the zero row.
  - The active frame range [0, T_a) is split across 8 cores, FS frames
    per core (multiple of 128). Each core:
      1. indirect-DMA gathers its FS frame-rows of d_pad and t_pad into
         SBUF ([128 frames x NCHUNK chunks] layout),
      2. PE-transposes 128x128 blocks (bit-exact) into PSUM,
      3. copies PSUM -> SBUF and DMAs [128, FS] slabs to its outputs.
  - Host places the 8 slabs into the full outputs; the tail past T_a
    stays zero (it is exactly zero mathematically).

Everything on device is data movement (DMA/transpose/copy) - results are
bit-exact fp32 copies of input values.
"""

import os
import numpy as np

MAX_FRAMES = 8192
N_CORES = 8
P = 128

# Results object of the last hardware run (exec_time_ns etc.), for the
# local test harness. The grader only uses the return value of kernel().
LAST_RESULTS = None


def _build_program(FS, NCHUNK, LP, C_D, C_T):
    import concourse.bacc as bacc
    import concourse.bass as bass
    import concourse.mybir as mybir
    import concourse.tile as tile
    from concourse.masks import make_identity

    nc = bacc.Bacc(
        "TRN2",
        target_bir_lowering=False,
        debug=False,
        num_devices=N_CORES,
    )

    NI16 = FS // 16  # dma_gather index columns (indices wrapped over 16 partitions)

    d_pad = nc.dram_tensor("d_pad", [LP, C_D], mybir.dt.float32, kind="ExternalInput")
    t_pad = nc.dram_tensor("t_pad", [LP, C_T], mybir.dt.float32, kind="ExternalInput")
    idx = nc.dram_tensor("idx", [P, NI16], mybir.dt.int16, kind="ExternalInput")
    en_out = nc.dram_tensor("en_out", [C_D, FS], mybir.dt.float32, kind="ExternalOutput")
    asr_out = nc.dram_tensor("asr_out", [C_T, FS], mybir.dt.float32, kind="ExternalOutput")

    MD = C_D // P  # en M-tiles (5)
    MT = C_T // P  # asr M-tiles (4)
    banks_per_tile = -(-(FS * 4) // 2048)  # PSUM bank = 2KB
    psum_bufs = max(1, 8 // banks_per_tile)

    with tile.TileContext(nc) as tc:
        with (
            tc.tile_pool(name="sbuf", bufs=1) as pool,
            tc.tile_pool(
                name="psum", bufs=psum_bufs, space=bass.MemorySpace.PSUM
            ) as psum_pool,
        ):
            ident = pool.tile([P, P], mybir.dt.float32)
            make_identity(nc, ident[:])

            idx_sb = pool.tile([P, NI16], mybir.dt.int16)
            nc.sync.dma_start(out=idx_sb[:], in_=idx[:])

            # Gather frame-rows: gath[i % 128, i // 128, :] = src[idx[i], :]
            gath_d = pool.tile([P, NCHUNK, C_D], mybir.dt.float32)
            nc.gpsimd.dma_gather(
                out_ap=gath_d[:],
                in_ap=d_pad[:],
                idxs_ap=idx_sb[:],
                num_idxs=FS,
                num_idxs_reg=FS,
                elem_size=C_D,
            )
            gath_t = pool.tile([P, NCHUNK, C_T], mybir.dt.float32)
            nc.gpsimd.dma_gather(
                out_ap=gath_t[:],
                in_ap=t_pad[:],
                idxs_ap=idx_sb[:],
                num_idxs=FS,
                num_idxs_reg=FS,
                elem_size=C_T,
            )

            # Transpose [frame, chan] blocks to [chan, frame] and write out.
            def emit(gath, n_mtiles, out_dram, sb_tile):
                for m in range(n_mtiles):
                    ps = psum_pool.tile([P, FS], mybir.dt.float32, tag="ps")
                    for j in range(NCHUNK):
                        nc.tensor.transpose(
                            ps[:, j * P : (j + 1) * P],
                            gath[:, j, m * P : (m + 1) * P],
                            ident[:],
                        )
                    nc.vector.tensor_copy(sb_tile[:, m, :], ps[:])
                    nc.sync.dma_start(
                        out=out_dram[m * P : (m + 1) * P, :],
                        in_=sb_tile[:, m, :],
                    )

            en_sb = pool.tile([P, MD, FS], mybir.dt.float32)
            asr_sb = pool.tile([P, MT, FS], mybir.dt.float32)
            emit(gath_d, MD, en_out, en_sb)
            emit(gath_t, MT, asr_out, asr_sb)

    nc.compile()
    return nc


def _prepare(pred_dur, d, t_en):
    """Host-side shard prep. Returns (FS, NCHUNK, in_maps) or None if T_a==0."""
    L = pred_dur.shape[1]
    C_D = d.shape[2]
    C_T = t_en.shape[1]

    dur = np.asarray(pred_dur[0], dtype=np.int64)
    cum = np.cumsum(dur)
    T_a = int(cum[-1])
    if T_a <= 0:
        return None

    FS = -(-T_a // N_CORES)  # ceil
    FS = -(-FS // P) * P  # round up to multiple of 128
    NCHUNK = FS // P

    # frame -> owning token; frames past T_a hit the zero row (index L)
    ind = np.searchsorted(cum, np.arange(T_a), side="right").astype(np.int32)
    idx_all = np.full(N_CORES * FS, L, dtype=np.int32)
    idx_all[:T_a] = ind

    d_pad = np.concatenate([d[0], np.zeros((1, C_D), np.float32)], axis=0)
    t_pad = np.concatenate(
        [np.ascontiguousarray(t_en[0].T), np.zeros((1, C_T), np.float32)], axis=0
    )

    in_maps = []
    NI16 = FS // 16
    for k in range(N_CORES):
        # dma_gather index layout: idx16[p, s] = index of frame s*16 + (p % 16)
        # (wrapped over 16 partitions and replicated across the 8 Q7 cores,
        # which read different partition groups)
        wrap = idx_all[k * FS : (k + 1) * FS].reshape(NI16, 16).T
        idx16 = np.ascontiguousarray(np.tile(wrap, (P // 16, 1)), dtype=np.int16)
        in_maps.append(
            {
                "d_pad": d_pad,
                "t_pad": t_pad,
                "idx": idx16,
            }
        )
    return FS, NCHUNK, in_maps


def kernel(pred_dur, d, t_en):
    global LAST_RESULTS

    pred_dur = np.asarray(pred_dur)
    d = np.asarray(d, dtype=np.float32)
    t_en = np.asarray(t_en, dtype=np.float32)
    B, L = pred_dur.shape
    assert B == 1
    C_D = d.shape[2]
    C_T = t_en.shape[1]

    en = np.zeros((B, C_D, MAX_FRAMES), np.float32)
    asr = np.zeros((B, C_T, MAX_FRAMES), np.float32)

    prep = _prepare(pred_dur, d, t_en)
    if prep is None:
        return en, asr
    FS, NCHUNK, in_maps = prep

    from concourse.bass_utils import run_bass_kernel_spmd

    nc = _build_program(FS, NCHUNK, L + 1, C_D, C_T)
    trace = bool(os.environ.get("KERNEL_TRACE"))
    res = run_bass_kernel_spmd(
        nc,
        in_maps,
        core_ids=list(range(N_CORES)),
        trace=trace,
        trace_cores=list(range(N_CORES)) if trace else None,
    )
    LAST_RESULTS = res

    for k in range(N_CORES):
        f0 = k * FS
        f1 = min(f0 + FS, MAX_FRAMES)
        en[0, :, f0:f1] = res.results[k]["en_out"][:, : f1 - f0]
        asr[0, :, f0:f1] = res.results[k]["asr_out"][:, : f1 - f0]
    return en, asr


# revision 10
# speedup vs baseline: 1.1447x; 1.1447x over previous
"""Trainium2 Bass kernel for CoreML alignment (duration -> frame expansion).

Math: each output frame f belongs to exactly one token l (the alignment
matrix is one-hot per frame: starts[l] <= f < cum_dur[l]), so

    en[:, c, f]  = d[:, tok(f), c]      (gather of d rows)
    asr[:, c, f] = t_en[:, c, tok(f)]   (gather of t_en columns)

and every column f >= T_a = sum(dur) is exactly zero.

Strategy (frame-sharded across 8 cores, no collectives):
  - Host computes cum_dur / frame->token indices (tiny metadata) and pads
    d / t_en^T with one zero row; inactive frames index the zero row.
  - The active frame range [0, T_a) is split across 8 cores, FS frames
    per core (multiple of 128). Each core:
      1. indirect-DMA gathers its frame-rows of d_pad and t_pad into SBUF,
         one 128-frame chunk per instruction (one index per partition),
      2. PE-transposes 128x128 blocks (bit-exact) into PSUM as each chunk
         lands,
      3. copies PSUM -> SBUF (split across Vector/Scalar engines) and DMAs
         [128, FS] slabs to its outputs.
  - Host places the 8 slabs into the full outputs; the tail past T_a
    stays zero (it is exactly zero mathematically).

Everything on device is data movement (DMA/transpose/copy) - results are
bit-exact fp32 copies of input values.
"""

import os
import numpy as np

MAX_FRAMES = 8192
N_CORES = 8
P = 128

# Results object of the last hardware run (exec_time_ns etc.), for the
# local test harness. The grader only uses the return value of kernel().
LAST_RESULTS = None


def _build_program(FS, NCHUNK, LP, C_D, C_T):
    import concourse.bacc as bacc
    import concourse.bass as bass
    import concourse.mybir as mybir
    import concourse.tile as tile

    nc = bacc.Bacc(
        "TRN2",
        target_bir_lowering=False,
        debug=False,
        num_devices=N_CORES,
    )

    d_pad = nc.dram_tensor("d_pad", [LP, C_D], mybir.dt.float32, kind="ExternalInput")
    t_pad = nc.dram_tensor("t_pad", [LP, C_T], mybir.dt.float32, kind="ExternalInput")
    idx = nc.dram_tensor("idx", [P, NCHUNK], mybir.dt.int32, kind="ExternalInput")
    ident_in = nc.dram_tensor("ident", [P, P], mybir.dt.float32, kind="ExternalInput")
    en_out = nc.dram_tensor("en_out", [C_D, FS], mybir.dt.float32, kind="ExternalOutput")
    asr_out = nc.dram_tensor("asr_out", [C_T, FS], mybir.dt.float32, kind="ExternalOutput")

    MD = C_D // P  # en M-tiles (5)
    MT = C_T // P  # asr M-tiles (4)
    banks_per_tile = -(-(FS * 4) // 2048)  # PSUM bank = 2KB
    psum_bufs = max(1, 8 // banks_per_tile)

    with tile.TileContext(nc) as tc:
        with (
            tc.tile_pool(name="sbuf", bufs=1) as pool,
            tc.tile_pool(
                name="psum", bufs=psum_bufs, space=bass.MemorySpace.PSUM
            ) as psum_pool,
        ):
            idx_sb = pool.tile([P, NCHUNK], mybir.dt.int32)
            nc.sync.dma_start(out=idx_sb[:], in_=idx[:])
            ident = pool.tile([P, P], mybir.dt.float32)
            nc.sync.dma_start(out=ident[:], in_=ident_in[:])

            # Gather 128 frame-rows per instruction (one index per partition):
            # gath[p, :] = src[idx[p, j], :]
            gath_d = []
            gath_t = []
            for j in range(NCHUNK):
                gd = pool.tile([P, C_D], mybir.dt.float32, tag=f"gd{j}")
                nc.gpsimd.indirect_dma_start(
                    out=gd[:],
                    out_offset=None,
                    in_=d_pad[:],
                    in_offset=bass.IndirectOffsetOnAxis(ap=idx_sb[:, j : j + 1], axis=0),
                )
                gath_d.append(gd)
                gt = pool.tile([P, C_T], mybir.dt.float32, tag=f"gt{j}")
                nc.gpsimd.indirect_dma_start(
                    out=gt[:],
                    out_offset=None,
                    in_=t_pad[:],
                    in_offset=bass.IndirectOffsetOnAxis(ap=idx_sb[:, j : j + 1], axis=0),
                )
                gath_t.append(gt)

            # PSUM accumulators: [chan 128, frame FS] per M-tile
            ps_en = [
                psum_pool.tile([P, FS], mybir.dt.float32, tag="ps", name=f"ps_en{m}")
                for m in range(MD)
            ]
            ps_asr = [
                psum_pool.tile([P, FS], mybir.dt.float32, tag="ps", name=f"ps_asr{m}")
                for m in range(MT)
            ]

            # Transpose each gathered [128 frames, 128 chans] block as its
            # chunk arrives.
            for j in range(NCHUNK):
                for m in range(MD):
                    nc.tensor.transpose(
                        ps_en[m][:, j * P : (j + 1) * P],
                        gath_d[j][:, m * P : (m + 1) * P],
                        ident[:],
                    )
                for m in range(MT):
                    nc.tensor.transpose(
                        ps_asr[m][:, j * P : (j + 1) * P],
                        gath_t[j][:, m * P : (m + 1) * P],
                        ident[:],
                    )

            # PSUM -> SBUF (Vector/Scalar split) -> DRAM
            en_sb = pool.tile([P, MD, FS], mybir.dt.float32)
            asr_sb = pool.tile([P, MT, FS], mybir.dt.float32)
            outs = [(ps_en[m], en_sb[:, m, :], en_out[m * P : (m + 1) * P, :]) for m in range(MD)]
            outs += [(ps_asr[m], asr_sb[:, m, :], asr_out[m * P : (m + 1) * P, :]) for m in range(MT)]
            for i, (ps, sb, dram) in enumerate(outs):
                if i % 3 == 2:
                    nc.scalar.copy(sb, ps[:])
                else:
                    nc.vector.tensor_copy(sb, ps[:])
                nc.sync.dma_start(out=dram, in_=sb)

    nc.compile()
    return nc


def _prepare(pred_dur, d, t_en):
    """Host-side shard prep. Returns (FS, NCHUNK, in_maps) or None if T_a==0."""
    L = pred_dur.shape[1]
    C_D = d.shape[2]
    C_T = t_en.shape[1]

    dur = np.asarray(pred_dur[0], dtype=np.int64)
    cum = np.cumsum(dur)
    T_a = int(cum[-1])
    if T_a <= 0:
        return None

    FS = -(-T_a // N_CORES)  # ceil
    FS = -(-FS // P) * P  # round up to multiple of 128
    NCHUNK = FS // P

    # frame -> owning token; frames past T_a hit the zero row (index L)
    ind = np.searchsorted(cum, np.arange(T_a), side="right").astype(np.int32)
    idx_all = np.full(N_CORES * FS, L, dtype=np.int32)
    idx_all[:T_a] = ind

    d_pad = np.concatenate([d[0], np.zeros((1, C_D), np.float32)], axis=0)
    t_pad = np.concatenate(
        [np.ascontiguousarray(t_en[0].T), np.zeros((1, C_T), np.float32)], axis=0
    )
    ident = np.eye(P, dtype=np.float32)

    in_maps = []
    for k in range(N_CORES):
        # idx[p, j] = token index of frame k*FS + j*128 + p
        idx_k = np.ascontiguousarray(
            idx_all[k * FS : (k + 1) * FS].reshape(NCHUNK, P).T
        )
        in_maps.append(
            {"d_pad": d_pad, "t_pad": t_pad, "idx": idx_k, "ident": ident}
        )
    return FS, NCHUNK, in_maps


def kernel(pred_dur, d, t_en):
    global LAST_RESULTS

    pred_dur = np.asarray(pred_dur)
    d = np.asarray(d, dtype=np.float32)
    t_en = np.asarray(t_en, dtype=np.float32)
    B, L = pred_dur.shape
    assert B == 1
    C_D = d.shape[2]
    C_T = t_en.shape[1]

    en = np.zeros((B, C_D, MAX_FRAMES), np.float32)
    asr = np.zeros((B, C_T, MAX_FRAMES), np.float32)

    prep = _prepare(pred_dur, d, t_en)
    if prep is None:
        return en, asr
    FS, NCHUNK, in_maps = prep

    from concourse.bass_utils import run_bass_kernel_spmd

    nc = _build_program(FS, NCHUNK, L + 1, C_D, C_T)
    trace = bool(os.environ.get("KERNEL_TRACE"))
    res = run_bass_kernel_spmd(
        nc,
        in_maps,
        core_ids=list(range(N_CORES)),
        trace=trace,
        trace_cores=list(range(N_CORES)) if trace else None,
    )
    LAST_RESULTS = res

    for k in range(N_CORES):
        f0 = k * FS
        f1 = min(f0 + FS, MAX_FRAMES)
        en[0, :, f0:f1] = res.results[k]["en_out"][:, : f1 - f0]
        asr[0, :, f0:f1] = res.results[k]["asr_out"][:, : f1 - f0]
    return en, asr


# revision 12
# speedup vs baseline: 1.1764x; 1.0277x over previous
"""Trainium2 Bass kernel for CoreML alignment (duration -> frame expansion).

Math: each output frame f belongs to exactly one token l (the alignment
matrix is one-hot per frame: starts[l] <= f < cum_dur[l]), so

    en[:, c, f]  = d[:, tok(f), c]      (gather of d rows)
    asr[:, c, f] = t_en[:, c, tok(f)]   (gather of t_en columns)

and every column f >= T_a = sum(dur) is exactly zero.

Strategy (frame-sharded across 8 cores, no collectives):
  - Host computes cum_dur / frame->token indices (tiny metadata) and pads
    d / t_en^T with one zero row; inactive frames index the zero row.
  - The active frame range [0, T_a) is split across 8 cores, FS frames
    per core (multiple of 128). Each core:
      1. indirect-DMA gathers its frame-rows of d_pad and t_pad into SBUF,
         one 128-frame chunk per instruction (one index per partition),
      2. PE-transposes 128x128 blocks (bit-exact) into PSUM as each chunk
         lands,
      3. copies PSUM -> SBUF (split across Vector/Scalar engines) and DMAs
         [128, FS] slabs to its outputs.
  - Host places the 8 slabs into the full outputs; the tail past T_a
    stays zero (it is exactly zero mathematically).

Everything on device is data movement (DMA/transpose/copy) - results are
bit-exact fp32 copies of input values.
"""

import os
import numpy as np

MAX_FRAMES = 8192
N_CORES = 8
P = 128

# Results object of the last hardware run (exec_time_ns etc.), for the
# local test harness. The grader only uses the return value of kernel().
LAST_RESULTS = None


def _build_program(FS, NCHUNK, LP, C_D, C_T):
    import concourse.bacc as bacc
    import concourse.bass as bass
    import concourse.mybir as mybir
    import concourse.tile as tile

    nc = bacc.Bacc(
        "TRN2",
        target_bir_lowering=False,
        debug=False,
        num_devices=N_CORES,
    )

    d_pad = nc.dram_tensor("d_pad", [LP, C_D], mybir.dt.float32, kind="ExternalInput")
    t_pad = nc.dram_tensor("t_pad", [LP, C_T], mybir.dt.float32, kind="ExternalInput")
    idx = nc.dram_tensor("idx", [P, NCHUNK], mybir.dt.int32, kind="ExternalInput")
    ident_in = nc.dram_tensor("ident", [P, P], mybir.dt.float32, kind="ExternalInput")
    en_out = nc.dram_tensor("en_out", [C_D, FS], mybir.dt.float32, kind="ExternalOutput")
    asr_out = nc.dram_tensor("asr_out", [C_T, FS], mybir.dt.float32, kind="ExternalOutput")

    MD = C_D // P  # en M-tiles (5)
    MT = C_T // P  # asr M-tiles (4)
    banks_per_tile = -(-(FS * 4) // 2048)  # PSUM bank = 2KB
    psum_bufs = max(1, 8 // banks_per_tile)

    with tile.TileContext(nc) as tc:
        with (
            tc.tile_pool(name="sbuf", bufs=1) as pool,
            tc.tile_pool(
                name="psum", bufs=psum_bufs, space=bass.MemorySpace.PSUM
            ) as psum_pool,
        ):
            # idx goes through GPSIMD (SWDGE): it is the gathers' own engine,
            # idle during the Tile prologue, so the load lands ~2.5us earlier
            # than a Sync-queue DMA would.
            idx_sb = pool.tile([P, NCHUNK], mybir.dt.int32)
            nc.gpsimd.dma_start(out=idx_sb[:], in_=idx[:])
            ident = pool.tile([P, P], mybir.dt.float32)
            nc.sync.dma_start(out=ident[:], in_=ident_in[:])

            # Gather 128 frame-rows per instruction (one index per partition):
            # gath[p, :] = src[idx[p, j], :]
            gath_d = []
            gath_t = []
            for j in range(NCHUNK):
                gd = pool.tile([P, C_D], mybir.dt.float32, tag=f"gd{j}")
                nc.gpsimd.indirect_dma_start(
                    out=gd[:],
                    out_offset=None,
                    in_=d_pad[:],
                    in_offset=bass.IndirectOffsetOnAxis(ap=idx_sb[:, j : j + 1], axis=0),
                )
                gath_d.append(gd)
                gt = pool.tile([P, C_T], mybir.dt.float32, tag=f"gt{j}")
                nc.gpsimd.indirect_dma_start(
                    out=gt[:],
                    out_offset=None,
                    in_=t_pad[:],
                    in_offset=bass.IndirectOffsetOnAxis(ap=idx_sb[:, j : j + 1], axis=0),
                )
                gath_t.append(gt)

            # PSUM accumulators: [chan 128, frame FS] per M-tile
            ps_en = [
                psum_pool.tile([P, FS], mybir.dt.float32, tag="ps", name=f"ps_en{m}")
                for m in range(MD)
            ]
            ps_asr = [
                psum_pool.tile([P, FS], mybir.dt.float32, tag="ps", name=f"ps_asr{m}")
                for m in range(MT)
            ]

            # Transpose each gathered [128 frames, 128 chans] block as its
            # chunk arrives.
            for j in range(NCHUNK):
                for m in range(MD):
                    nc.tensor.transpose(
                        ps_en[m][:, j * P : (j + 1) * P],
                        gath_d[j][:, m * P : (m + 1) * P],
                        ident[:],
                    )
                for m in range(MT):
                    nc.tensor.transpose(
                        ps_asr[m][:, j * P : (j + 1) * P],
                        gath_t[j][:, m * P : (m + 1) * P],
                        ident[:],
                    )

            # PSUM -> SBUF (Vector/Scalar split) -> DRAM
            en_sb = pool.tile([P, MD, FS], mybir.dt.float32)
            asr_sb = pool.tile([P, MT, FS], mybir.dt.float32)
            outs = [(ps_en[m], en_sb[:, m, :], en_out[m * P : (m + 1) * P, :]) for m in range(MD)]
            outs += [(ps_asr[m], asr_sb[:, m, :], asr_out[m * P : (m + 1) * P, :]) for m in range(MT)]
            for i, (ps, sb, dram) in enumerate(outs):
                if i % 3 == 2:
                    nc.scalar.copy(sb, ps[:])
                else:
                    nc.vector.tensor_copy(sb, ps[:])
                # alternate between the two HWDGE rings (SP / Activation) so
                # the nine output DMAs don't serialize on one FIFO
                dma_eng = nc.sync if i % 2 == 0 else nc.scalar
                dma_eng.dma_start(out=dram, in_=sb)

    nc.compile()
    return nc


def _prepare(pred_dur, d, t_en):
    """Host-side shard prep. Returns (FS, NCHUNK, in_maps) or None if T_a==0."""
    L = pred_dur.shape[1]
    C_D = d.shape[2]
    C_T = t_en.shape[1]

    dur = np.asarray(pred_dur[0], dtype=np.int64)
    cum = np.cumsum(dur)
    T_a = int(cum[-1])
    if T_a <= 0:
        return None

    FS = -(-T_a // N_CORES)  # ceil
    FS = -(-FS // P) * P  # round up to multiple of 128
    NCHUNK = FS // P

    # frame -> owning token; frames past T_a hit the zero row (index L)
    ind = np.searchsorted(cum, np.arange(T_a), side="right").astype(np.int32)
    idx_all = np.full(N_CORES * FS, L, dtype=np.int32)
    idx_all[:T_a] = ind

    d_pad = np.concatenate([d[0], np.zeros((1, C_D), np.float32)], axis=0)
    t_pad = np.concatenate(
        [np.ascontiguousarray(t_en[0].T), np.zeros((1, C_T), np.float32)], axis=0
    )
    ident = np.eye(P, dtype=np.float32)

    in_maps = []
    for k in range(N_CORES):
        # idx[p, j] = token index of frame k*FS + j*128 + p
        idx_k = np.ascontiguousarray(
            idx_all[k * FS : (k + 1) * FS].reshape(NCHUNK, P).T
        )
        in_maps.append(
            {"d_pad": d_pad, "t_pad": t_pad, "idx": idx_k, "ident": ident}
        )
    return FS, NCHUNK, in_maps


def kernel(pred_dur, d, t_en):
    global LAST_RESULTS

    pred_dur = np.asarray(pred_dur)
    d = np.asarray(d, dtype=np.float32)
    t_en = np.asarray(t_en, dtype=np.float32)
    B, L = pred_dur.shape
    assert B == 1
    C_D = d.shape[2]
    C_T = t_en.shape[1]

    en = np.zeros((B, C_D, MAX_FRAMES), np.float32)
    asr = np.zeros((B, C_T, MAX_FRAMES), np.float32)

    prep = _prepare(pred_dur, d, t_en)
    if prep is None:
        return en, asr
    FS, NCHUNK, in_maps = prep

    from concourse.bass_utils import run_bass_kernel_spmd

    nc = _build_program(FS, NCHUNK, L + 1, C_D, C_T)
    trace = bool(os.environ.get("KERNEL_TRACE"))
    res = run_bass_kernel_spmd(
        nc,
        in_maps,
        core_ids=list(range(N_CORES)),
        trace=trace,
        trace_cores=list(range(N_CORES)) if trace else None,
    )
    LAST_RESULTS = res

    for k in range(N_CORES):
        f0 = k * FS
        f1 = min(f0 + FS, MAX_FRAMES)
        en[0, :, f0:f1] = res.results[k]["en_out"][:, : f1 - f0]
        asr[0, :, f0:f1] = res.results[k]["asr_out"][:, : f1 - f0]
    return en, asr


# revision 14
# speedup vs baseline: 1.2799x; 1.0879x over previous
"""Trainium2 Bass kernel for CoreML alignment (duration -> frame expansion).

Math: each output frame f belongs to exactly one token l (the alignment
matrix is one-hot per frame: starts[l] <= f < cum_dur[l]), so

    en[:, c, f]  = d[:, tok(f), c]      (gather of d rows)
    asr[:, c, f] = t_en[:, c, tok(f)]   (gather of t_en columns)

and every column f >= T_a = sum(dur) is exactly zero.

Strategy (frame-sharded across 8 cores, no collectives):
  - Host computes cum_dur / frame->token indices (tiny metadata) and pads
    d / t_en^T with one zero row; inactive frames index the zero row.
  - The active frame range [0, T_a) is split across 8 cores, FS frames
    per core (multiple of 128). Each core:
      1. indirect-DMA gathers its frame-rows of d_pad and t_pad into SBUF,
         one 128-frame chunk per instruction (one index per partition),
      2. PE-transposes 128x128 blocks (bit-exact) into PSUM as each chunk
         lands,
      3. copies PSUM -> SBUF (split across Vector/Scalar engines) and DMAs
         [128, FS] slabs to its outputs.
  - Host places the 8 slabs into the full outputs; the tail past T_a
    stays zero (it is exactly zero mathematically).

Everything on device is data movement (DMA/transpose/copy) - results are
bit-exact fp32 copies of input values.
"""

import os
import numpy as np

MAX_FRAMES = 8192
N_CORES = 8
P = 128

# Results object of the last hardware run (exec_time_ns etc.), for the
# local test harness. The grader only uses the return value of kernel().
LAST_RESULTS = None


def _build_program(FS, NCHUNK, LP, C_D, C_T):
    import concourse.bacc as bacc
    import concourse.bass as bass
    import concourse.mybir as mybir
    import concourse.tile as tile

    nc = bacc.Bacc(
        "TRN2",
        target_bir_lowering=False,
        debug=False,
        num_devices=N_CORES,
    )

    d_pad = nc.dram_tensor("d_pad", [LP, C_D], mybir.dt.float32, kind="ExternalInput")
    t_pad = nc.dram_tensor("t_pad", [LP, C_T], mybir.dt.float32, kind="ExternalInput")
    idx = nc.dram_tensor("idx", [P, NCHUNK], mybir.dt.int32, kind="ExternalInput")
    ident_in = nc.dram_tensor("ident", [P, P], mybir.dt.float32, kind="ExternalInput")
    en_out = nc.dram_tensor("en_out", [C_D, FS], mybir.dt.float32, kind="ExternalOutput")
    asr_out = nc.dram_tensor("asr_out", [C_T, FS], mybir.dt.float32, kind="ExternalOutput")

    MD = C_D // P  # en M-tiles (5)
    MT = C_T // P  # asr M-tiles (4)
    banks_per_tile = -(-(FS * 4) // 2048)  # PSUM bank = 2KB
    psum_bufs = max(1, 8 // banks_per_tile)

    with tile.TileContext(nc) as tc:
        with (
            tc.tile_pool(name="sbuf", bufs=1) as pool,
            tc.tile_pool(
                name="psum", bufs=psum_bufs, space=bass.MemorySpace.PSUM
            ) as psum_pool,
        ):
            idx_sb = pool.tile([P, NCHUNK], mybir.dt.int32)
            nc.sync.dma_start(out=idx_sb[:], in_=idx[:])
            ident = pool.tile([P, P], mybir.dt.float32)
            nc.sync.dma_start(out=ident[:], in_=ident_in[:])

            # Gather 128 frame-rows per instruction (one index per partition):
            # gath[p, :] = src[idx[p, j], :]. All d chunks first: en tiles
            # then complete (transpose+copy+store) while the t gathers are
            # still running, and their freed PSUM banks cover the 9th tile.
            gath_d = []
            gath_t = []
            for j in range(NCHUNK):
                gd = pool.tile([P, C_D], mybir.dt.float32, tag=f"gd{j}")
                nc.gpsimd.indirect_dma_start(
                    out=gd[:],
                    out_offset=None,
                    in_=d_pad[:],
                    in_offset=bass.IndirectOffsetOnAxis(ap=idx_sb[:, j : j + 1], axis=0),
                )
                gath_d.append(gd)
            for j in range(NCHUNK):
                gt = pool.tile([P, C_T], mybir.dt.float32, tag=f"gt{j}")
                nc.gpsimd.indirect_dma_start(
                    out=gt[:],
                    out_offset=None,
                    in_=t_pad[:],
                    in_offset=bass.IndirectOffsetOnAxis(ap=idx_sb[:, j : j + 1], axis=0),
                )
                gath_t.append(gt)

            # PSUM accumulators: [chan 128, frame FS] per M-tile
            ps_en = [
                psum_pool.tile([P, FS], mybir.dt.float32, tag="ps", name=f"ps_en{m}")
                for m in range(MD)
            ]
            ps_asr = [
                psum_pool.tile([P, FS], mybir.dt.float32, tag="ps", name=f"ps_asr{m}")
                for m in range(MT)
            ]

            en_sb = pool.tile([P, MD, FS], mybir.dt.float32)
            asr_sb = pool.tile([P, MT, FS], mybir.dt.float32)

            # Transpose each gathered [128 frames, 128 chans] block as its
            # chunk arrives; copy/store each M-tile as soon as its last
            # chunk is transposed. Copies alternate Vector/Scalar; output
            # DMAs alternate the two HWDGE rings (SP / Activation).
            def emit(gath, ps_list, sb, out_dram, n_mtiles, flip):
                for j in range(NCHUNK):
                    for m in range(n_mtiles):
                        nc.tensor.transpose(
                            ps_list[m][:, j * P : (j + 1) * P],
                            gath[j][:, m * P : (m + 1) * P],
                            ident[:],
                        )
                for m in range(n_mtiles):
                    if (m + flip) % 2 == 0:
                        nc.vector.tensor_copy(sb[:, m, :], ps_list[m][:])
                        nc.sync.dma_start(
                            out=out_dram[m * P : (m + 1) * P, :], in_=sb[:, m, :]
                        )
                    else:
                        nc.scalar.copy(sb[:, m, :], ps_list[m][:])
                        nc.scalar.dma_start(
                            out=out_dram[m * P : (m + 1) * P, :], in_=sb[:, m, :]
                        )

            emit(gath_d, ps_en, en_sb, en_out, MD, 0)
            emit(gath_t, ps_asr, asr_sb, asr_out, MT, 1)

    nc.compile()
    return nc


def _prepare(pred_dur, d, t_en):
    """Host-side shard prep. Returns (FS, NCHUNK, in_maps) or None if T_a==0."""
    L = pred_dur.shape[1]
    C_D = d.shape[2]
    C_T = t_en.shape[1]

    dur = np.asarray(pred_dur[0], dtype=np.int64)
    cum = np.cumsum(dur)
    T_a = int(cum[-1])
    if T_a <= 0:
        return None

    FS = -(-T_a // N_CORES)  # ceil
    FS = -(-FS // P) * P  # round up to multiple of 128
    NCHUNK = FS // P

    # frame -> owning token; frames past T_a hit the zero row (index L)
    ind = np.searchsorted(cum, np.arange(T_a), side="right").astype(np.int32)
    idx_all = np.full(N_CORES * FS, L, dtype=np.int32)
    idx_all[:T_a] = ind

    d_pad = np.concatenate([d[0], np.zeros((1, C_D), np.float32)], axis=0)
    t_pad = np.concatenate(
        [np.ascontiguousarray(t_en[0].T), np.zeros((1, C_T), np.float32)], axis=0
    )
    ident = np.eye(P, dtype=np.float32)

    in_maps = []
    for k in range(N_CORES):
        # idx[p, j] = token index of frame k*FS + j*128 + p
        idx_k = np.ascontiguousarray(
            idx_all[k * FS : (k + 1) * FS].reshape(NCHUNK, P).T
        )
        in_maps.append(
            {"d_pad": d_pad, "t_pad": t_pad, "idx": idx_k, "ident": ident}
        )
    return FS, NCHUNK, in_maps


def kernel(pred_dur, d, t_en):
    global LAST_RESULTS

    pred_dur = np.asarray(pred_dur)
    d = np.asarray(d, dtype=np.float32)
    t_en = np.asarray(t_en, dtype=np.float32)
    B, L = pred_dur.shape
    assert B == 1
    C_D = d.shape[2]
    C_T = t_en.shape[1]

    en = np.zeros((B, C_D, MAX_FRAMES), np.float32)
    asr = np.zeros((B, C_T, MAX_FRAMES), np.float32)

    prep = _prepare(pred_dur, d, t_en)
    if prep is None:
        return en, asr
    FS, NCHUNK, in_maps = prep

    from concourse.bass_utils import run_bass_kernel_spmd

    nc = _build_program(FS, NCHUNK, L + 1, C_D, C_T)
    trace = bool(os.environ.get("KERNEL_TRACE"))
    res = run_bass_kernel_spmd(
        nc,
        in_maps,
        core_ids=list(range(N_CORES)),
        trace=trace,
        trace_cores=list(range(N_CORES)) if trace else None,
    )
    LAST_RESULTS = res

    for k in range(N_CORES):
        f0 = k * FS
        f1 = min(f0 + FS, MAX_FRAMES)
        en[0, :, f0:f1] = res.results[k]["en_out"][:, : f1 - f0]
        asr[0, :, f0:f1] = res.results[k]["asr_out"][:, : f1 - f0]
    return en, asr
